# revision 2
# baseline (speedup 1.0000x reference)
"""Trainium2 Bass kernel for nn_Attention_21088289423660 (sparse_attention).

Reference computation (per token t = (b, n, m), feature dim D=256):
    kh = Wk^T k_t                  (feature-major: [e, t])
    qh = Wq^T q_t
    v  = Wv^T kh = (Wk Wv)^T k_t   <- folded on host: Wkv = Wk @ Wv
    S  = kh - qh + pos_t           <- Wqn = -Wq accumulated in PSUM
    attn = sigmoid(W2^T relu(W1^T S + b1) + b2)      (mask is all-ones)
    out  = Wo^T ((v + pos_t) * attn) (+ bo on host)  (already feature-major)

Sharding: data-parallel over 8 cores; core c handles batch b=c//2 and
N-half (c%2) -> 16384 tokens/core, weights replicated.

Compute dtype: bf16 (PSUM accumulation fp32), device output bf16,
host adds bo and widens to fp32.
"""

import os
import sys

for _p in (
    "/root/.axon_site",
    "/root/.axon_site/_ro/trn_rl_repo",
    "/root/.axon_site/_ro/pypackages",
    "/opt/trn_rl_repo",
):
    if os.path.isdir(_p) and _p not in sys.path:
        sys.path.append(_p)

import numpy as np
import ml_dtypes
from contextlib import ExitStack

import concourse.bass as bass
import concourse.tile as tile
import concourse.bacc as bacc
from concourse import mybir
from concourse import bass_utils

BF16 = ml_dtypes.bfloat16

B, DIM, N, M = 4, 256, 4096, 8
NCORES = 8
NT = (B * N * M) // NCORES          # tokens per core = 16384
P = 128                              # partitions
NDC = DIM // P                       # d-chunks = 2
CHUNK = 4096                         # tokens per DMA chunk
FD = 512                             # tokens per matmul tile
F32 = mybir.dt.float32
BF = mybir.dt.bfloat16

_CACHED_NC = None


def _build_nc():
    """Build and compile the per-core Bass program (SPMD, identical on all cores)."""
    nc = bacc.Bacc("TRN2", target_bir_lowering=False, debug=False)

    # DRAM I/O (per-core shapes)
    q_d = nc.dram_tensor("qs", (NDC, P, NT), BF, kind="ExternalInput").ap()
    k_d = nc.dram_tensor("ks", (NDC, P, NT), BF, kind="ExternalInput").ap()
    pos_d = nc.dram_tensor("poss", (NDC, P, NT), BF, kind="ExternalInput").ap()
    wk_d = nc.dram_tensor("wk", (NDC, P, DIM), BF, kind="ExternalInput").ap()
    wqn_d = nc.dram_tensor("wqn", (NDC, P, DIM), BF, kind="ExternalInput").ap()
    wkv_d = nc.dram_tensor("wkv", (NDC, P, DIM), BF, kind="ExternalInput").ap()
    w1_d = nc.dram_tensor("w1", (NDC, P, DIM // 2), BF, kind="ExternalInput").ap()
    w2_d = nc.dram_tensor("w2", (P, DIM), BF, kind="ExternalInput").ap()
    wo_d = nc.dram_tensor("wo", (NDC, P, DIM), BF, kind="ExternalInput").ap()
    b1_d = nc.dram_tensor("b1", (P, 1), F32, kind="ExternalInput").ap()
    b2_d = nc.dram_tensor("b2", (NDC, P, 1), F32, kind="ExternalInput").ap()
    out_d = nc.dram_tensor("out", (NDC, P, NT), BF, kind="ExternalOutput").ap()

    q_r = q_d.rearrange("c p t -> p c t")
    k_r = k_d.rearrange("c p t -> p c t")
    pos_r = pos_d.rearrange("c p t -> p c t")
    out_r = out_d.rearrange("c p t -> p c t")

    AF = mybir.ActivationFunctionType

    with tile.TileContext(nc) as tc, ExitStack() as ctx:
        wpool = ctx.enter_context(tc.tile_pool(name="wpool", bufs=1))
        iopool = ctx.enter_context(tc.tile_pool(name="iopool", bufs=2))
        mid = ctx.enter_context(tc.tile_pool(name="mid", bufs=3))
        pp = ctx.enter_context(tc.tile_pool(name="pp", bufs=1, space="PSUM"))

        # --- weights / biases resident in SBUF ---
        wk_t, wqn_t, wkv_t, wo_t, w1_t = [], [], [], [], []
        for c in range(NDC):
            wt = wpool.tile([P, DIM], BF, tag=f"wk{c}", name=f"wk{c}")
            nc.sync.dma_start(wt[:], wk_d[c])
            wk_t.append(wt)
            wt = wpool.tile([P, DIM], BF, tag=f"wqn{c}", name=f"wqn{c}")
            nc.sync.dma_start(wt[:], wqn_d[c])
            wqn_t.append(wt)
            wt = wpool.tile([P, DIM], BF, tag=f"wkv{c}", name=f"wkv{c}")
            nc.sync.dma_start(wt[:], wkv_d[c])
            wkv_t.append(wt)
            wt = wpool.tile([P, DIM], BF, tag=f"wo{c}", name=f"wo{c}")
            nc.sync.dma_start(wt[:], wo_d[c])
            wo_t.append(wt)
            wt = wpool.tile([P, DIM // 2], BF, tag=f"w1{c}", name=f"w1{c}")
            nc.sync.dma_start(wt[:], w1_d[c])
            w1_t.append(wt)
        w2_t = wpool.tile([P, DIM], BF, tag="w2", name="w2")
        nc.sync.dma_start(w2_t[:], w2_d[:])
        b1_t = wpool.tile([P, 1], F32, tag="b1", name="b1")
        nc.sync.dma_start(b1_t[:], b1_d[:])
        b2_t = []
        for c in range(NDC):
            bt = wpool.tile([P, 1], F32, tag=f"b2{c}", name=f"b2{c}")
            nc.sync.dma_start(bt[:], b2_d[c])
            b2_t.append(bt)

        n_chunks = NT // CHUNK
        n_iters = CHUNK // FD
        for ci in range(n_chunks):
            csl = bass.ts(ci, CHUNK)
            qt = iopool.tile([P, NDC, CHUNK], BF, tag="qt", name="qt")
            kt = iopool.tile([P, NDC, CHUNK], BF, tag="kt", name="kt")
            post = iopool.tile([P, NDC, CHUNK], BF, tag="post", name="post")
            nc.sync.dma_start(qt[:], q_r[:, :, csl])
            nc.sync.dma_start(kt[:], k_r[:, :, csl])
            nc.sync.dma_start(post[:], pos_r[:, :, csl])
            outt = iopool.tile([P, NDC, CHUNK], BF, tag="outt", name="outt")

            for it in range(n_iters):
                tsl = bass.ts(it, FD)
                # S[e,t] = kh - qh  (2-bank PSUM tile, 8 accumulating MMs)
                S = pp.tile([P, NDC, FD], F32, tag="S", name="S")
                for e in range(NDC):
                    esl = bass.ts(e, P)
                    nc.tensor.matmul(S[:, e, :], wk_t[0][:, esl], kt[:, 0, tsl],
                                     start=True, stop=False)
                    nc.tensor.matmul(S[:, e, :], wk_t[1][:, esl], kt[:, 1, tsl],
                                     start=False, stop=False)
                    nc.tensor.matmul(S[:, e, :], wqn_t[0][:, esl], qt[:, 0, tsl],
                                     start=False, stop=False)
                    nc.tensor.matmul(S[:, e, :], wqn_t[1][:, esl], qt[:, 1, tsl],
                                     start=False, stop=True)
                # v[e,t] = Wkv^T k  (2-bank PSUM tile)
                V = pp.tile([P, NDC, FD], F32, tag="v", name="V")
                for e in range(NDC):
                    esl = bass.ts(e, P)
                    nc.tensor.matmul(V[:, e, :], wkv_t[0][:, esl], kt[:, 0, tsl],
                                     start=True, stop=False)
                    nc.tensor.matmul(V[:, e, :], wkv_t[1][:, esl], kt[:, 1, tsl],
                                     start=False, stop=True)
                # attn_pre = S + pos ; vplus = v + pos  (single wide DVE ops)
                ap_t = mid.tile([P, NDC, FD], BF, tag="ap", name="ap_t")
                nc.vector.tensor_add(ap_t[:], S[:], post[:, :, tsl])
                vp_t = mid.tile([P, NDC, FD], BF, tag="vp", name="vp_t")
                nc.vector.tensor_add(vp_t[:], V[:], post[:, :, tsl])
                # h1 = relu(W1^T attn_pre + b1)
                h1p = pp.tile([P, FD], F32, tag="mlp", bufs=2, name="h1p")
                nc.tensor.matmul(h1p[:], w1_t[0][:], ap_t[:, 0, :], start=True, stop=False)
                nc.tensor.matmul(h1p[:], w1_t[1][:], ap_t[:, 1, :], start=False, stop=True)
                h1r = mid.tile([P, FD], BF, tag="h1r", name="h1r")
                nc.scalar.activation(h1r[:], h1p[:], AF.Relu, bias=b1_t[:, 0:1])
                # attn = sigmoid(W2^T h1 + b2)
                at_t = mid.tile([P, NDC, FD], BF, tag="at", name="at_t")
                for e in range(NDC):
                    esl = bass.ts(e, P)
                    a2p = pp.tile([P, FD], F32, tag="mlp", bufs=2, name="a2p")
                    nc.tensor.matmul(a2p[:], w2_t[:, esl], h1r[:], start=True, stop=True)
                    nc.scalar.activation(at_t[:, e, :], a2p[:], AF.Sigmoid,
                                         bias=b2_t[e][:, 0:1])
                # g = (v+pos)*attn  (GpSimd, SBUF-only)
                g_t = mid.tile([P, NDC, FD], BF, tag="g", name="g_t")
                nc.gpsimd.tensor_mul(g_t[:], vp_t[:], at_t[:])
                # out = Wo^T g  (bo added on host)
                xo = pp.tile([P, NDC, FD], F32, tag="xo", name="xo")
                for e in range(NDC):
                    esl = bass.ts(e, P)
                    nc.tensor.matmul(xo[:, e, :], wo_t[0][:, esl], g_t[:, 0, :],
                                     start=True, stop=False)
                    nc.tensor.matmul(xo[:, e, :], wo_t[1][:, esl], g_t[:, 1, :],
                                     start=False, stop=True)
                nc.vector.tensor_copy(outt[:, 0, tsl], xo[:, 0, :])
                nc.scalar.copy(outt[:, 1, tsl], xo[:, 1, :])

            nc.sync.dma_start(out_r[:, :, csl], outt[:])

    nc.compile()
    return nc


def _get_nc():
    global _CACHED_NC
    if _CACHED_NC is None:
        _CACHED_NC = _build_nc()
    return _CACHED_NC


def _prep_in_maps(q, k, pos, Wq, Wk, Wv, W1, b1, W2, b2, Wo, bo):
    q = np.asarray(q, dtype=np.float32)
    k = np.asarray(k, dtype=np.float32)
    pos = np.asarray(pos, dtype=np.float32)

    weights = {
        "wk": np.ascontiguousarray(np.asarray(Wk, np.float32).astype(BF16)).reshape(NDC, P, DIM),
        "wqn": np.ascontiguousarray((-np.asarray(Wq, np.float32)).astype(BF16)).reshape(NDC, P, DIM),
        "wkv": np.ascontiguousarray(
            (np.asarray(Wk, np.float32) @ np.asarray(Wv, np.float32)).astype(BF16)
        ).reshape(NDC, P, DIM),
        "w1": np.ascontiguousarray(np.asarray(W1, np.float32).astype(BF16)).reshape(NDC, P, DIM // 2),
        "w2": np.ascontiguousarray(np.asarray(W2, np.float32).astype(BF16)).reshape(P, DIM),
        "wo": np.ascontiguousarray(np.asarray(Wo, np.float32).astype(BF16)).reshape(NDC, P, DIM),
        "b1": np.asarray(b1, np.float32).reshape(P, 1),
        "b2": np.asarray(b2, np.float32).reshape(NDC, P, 1),
    }

    nhalf = N // 2
    in_maps = []
    for c in range(NCORES):
        b = c // 2
        n0 = (c % 2) * nhalf
        qs = q[b, :, n0:n0 + nhalf, :].reshape(DIM, NT).astype(BF16)
        ks = k[b, :, n0:n0 + nhalf, :].reshape(DIM, NT).astype(BF16)
        ps = np.ascontiguousarray(
            pos[b, n0:n0 + nhalf].reshape(NT, DIM).T
        ).astype(BF16)
        m = dict(weights)
        m["qs"] = qs.reshape(NDC, P, NT)
        m["ks"] = ks.reshape(NDC, P, NT)
        m["poss"] = ps.reshape(NDC, P, NT)
        in_maps.append(m)
    return in_maps


def _run(in_maps, trace=False, **kwargs):
    nc = _get_nc()
    return bass_utils.run_bass_kernel_spmd(
        nc, in_maps, core_ids=list(range(NCORES)), trace=trace, **kwargs
    )


def _assemble(results, bo, mask):
    bo = np.asarray(bo, np.float32)
    out = np.empty((B, DIM, N, M), dtype=np.float32)
    nhalf = N // 2
    for c in range(NCORES):
        b = c // 2
        n0 = (c % 2) * nhalf
        r = results[c]["out"].reshape(DIM, nhalf, M).astype(np.float32)
        r += bo[:, None, None]
        out[b, :, n0:n0 + nhalf, :] = r
    mask = np.asarray(mask)
    if not np.all(mask != 0):
        # masked positions: sigmoid(-1e9)=0 -> x=0 -> out = bo
        zb, zn, zm = np.nonzero(mask[..., 0] == 0)
        out[zb, :, zn, zm] = bo[None, :]
    return out


def kernel(q, k, pos, mask, Wq, Wk, Wv, W1, b1, W2, b2, Wo, bo):
    in_maps = _prep_in_maps(q, k, pos, Wq, Wk, Wv, W1, b1, W2, b2, Wo, bo)
    res = _run(in_maps)
    return _assemble(res.results, bo, mask)


def install_profile_hook():
    """Register the axon NTFF profiling hook (antenv.axon_hooks shim) so
    run_bass_kernel_spmd(trace=True) yields exec_time_ns + perfetto trace."""
    import types

    try:
        import antenv.axon_hooks  # noqa: F401
        return True
    except ImportError:
        pass
    try:
        from trn_agent_boot.trn_boot import _ntff_profile_via_ctypes
    except ImportError:
        return False
    hook = _ntff_profile_via_ctypes("/opt/axon/libaxon_pjrt.so")
    if hook is None:
        return False
    mod = types.ModuleType("antenv.axon_hooks")
    mod.get_axon_ntff_profile_hook = lambda: hook
    mod.set_axon_ntff_profile_hook = lambda h: None
    import antenv

    sys.modules["antenv.axon_hooks"] = mod
    antenv.axon_hooks = mod
    # artifact upload has no share reachable from this container
    bass_utils.upload_artifacts = lambda tmpdir: tmpdir
    return True


# revision 3
# speedup vs baseline: 1.1476x; 1.1476x over previous
"""Trainium2 Bass kernel for nn_Attention_21088289423660 (sparse_attention).

Reference computation (per token t = (b, n, m), feature dim D=256):
    kh = Wk^T k_t                  (feature-major: [e, t])
    qh = Wq^T q_t
    v  = Wv^T kh = (Wk Wv)^T k_t   <- folded on host: Wkv = Wk @ Wv
    S  = kh - qh + pos_t           <- Wqn = -Wq accumulated in PSUM
    attn = sigmoid(W2^T relu(W1^T S + b1) + b2)      (mask is all-ones)
    out  = Wo^T ((v + pos_t) * attn) (+ bo on host)  (already feature-major)

Sharding: data-parallel over 8 cores; core c handles batch b=c//2 and
N-half (c%2) -> 16384 tokens/core, weights replicated.

Compute dtype: bf16 (PSUM accumulation fp32), device output bf16,
host adds bo and widens to fp32.
"""

import os
import sys

for _p in (
    "/root/.axon_site",
    "/root/.axon_site/_ro/trn_rl_repo",
    "/root/.axon_site/_ro/pypackages",
    "/opt/trn_rl_repo",
):
    if os.path.isdir(_p) and _p not in sys.path:
        sys.path.append(_p)

import numpy as np
import ml_dtypes
from contextlib import ExitStack

import concourse.bass as bass
import concourse.tile as tile
import concourse.bacc as bacc
from concourse import mybir
from concourse import bass_utils

BF16 = ml_dtypes.bfloat16

B, DIM, N, M = 4, 256, 4096, 8
NCORES = 8
NT = (B * N * M) // NCORES          # tokens per core = 16384
P = 128                              # partitions
NDC = DIM // P                       # d-chunks = 2
CHUNK = 4096                         # tokens per DMA chunk
FD = 512                             # tokens per matmul tile
F32 = mybir.dt.float32
BF = mybir.dt.bfloat16

_CACHED_NC = None


def _build_nc():
    """Build and compile the per-core Bass program (SPMD, identical on all cores)."""
    nc = bacc.Bacc("TRN2", target_bir_lowering=False, debug=False)

    # DRAM I/O (per-core shapes)
    q_d = nc.dram_tensor("qs", (NDC, P, NT), BF, kind="ExternalInput").ap()
    k_d = nc.dram_tensor("ks", (NDC, P, NT), BF, kind="ExternalInput").ap()
    pos_d = nc.dram_tensor("poss", (NDC, P, NT), BF, kind="ExternalInput").ap()
    wk_d = nc.dram_tensor("wk", (NDC, P, DIM), BF, kind="ExternalInput").ap()
    wqn_d = nc.dram_tensor("wqn", (NDC, P, DIM), BF, kind="ExternalInput").ap()
    wkv_d = nc.dram_tensor("wkv", (NDC, P, DIM), BF, kind="ExternalInput").ap()
    w1_d = nc.dram_tensor("w1", (NDC, P, DIM // 2), BF, kind="ExternalInput").ap()
    w2_d = nc.dram_tensor("w2", (P, DIM), BF, kind="ExternalInput").ap()
    wo_d = nc.dram_tensor("wo", (NDC, P, DIM), BF, kind="ExternalInput").ap()
    b1_d = nc.dram_tensor("b1", (P, 1), F32, kind="ExternalInput").ap()
    b2_d = nc.dram_tensor("b2", (NDC, P, 1), F32, kind="ExternalInput").ap()
    out_d = nc.dram_tensor("out", (NDC, P, NT), BF, kind="ExternalOutput").ap()

    q_r = q_d.rearrange("c p t -> p c t")
    k_r = k_d.rearrange("c p t -> p c t")
    pos_r = pos_d.rearrange("c p t -> p c t")
    out_r = out_d.rearrange("c p t -> p c t")

    AF = mybir.ActivationFunctionType

    with tile.TileContext(nc) as tc, ExitStack() as ctx:
        wpool = ctx.enter_context(tc.tile_pool(name="wpool", bufs=1))
        iopool = ctx.enter_context(tc.tile_pool(name="iopool", bufs=2))
        mid = ctx.enter_context(tc.tile_pool(name="mid", bufs=3))
        pp = ctx.enter_context(tc.tile_pool(name="pp", bufs=1, space="PSUM"))

        # --- weights / biases resident in SBUF ---
        wk_t, wqn_t, wkv_t, wo_t, w1_t = [], [], [], [], []
        for c in range(NDC):
            wt = wpool.tile([P, DIM], BF, tag=f"wk{c}", name=f"wk{c}")
            nc.sync.dma_start(wt[:], wk_d[c])
            wk_t.append(wt)
            wt = wpool.tile([P, DIM], BF, tag=f"wqn{c}", name=f"wqn{c}")
            nc.sync.dma_start(wt[:], wqn_d[c])
            wqn_t.append(wt)
            wt = wpool.tile([P, DIM], BF, tag=f"wkv{c}", name=f"wkv{c}")
            nc.sync.dma_start(wt[:], wkv_d[c])
            wkv_t.append(wt)
            wt = wpool.tile([P, DIM], BF, tag=f"wo{c}", name=f"wo{c}")
            nc.sync.dma_start(wt[:], wo_d[c])
            wo_t.append(wt)
            wt = wpool.tile([P, DIM // 2], BF, tag=f"w1{c}", name=f"w1{c}")
            nc.sync.dma_start(wt[:], w1_d[c])
            w1_t.append(wt)
        w2_t = wpool.tile([P, DIM], BF, tag="w2", name="w2")
        nc.sync.dma_start(w2_t[:], w2_d[:])
        b1_t = wpool.tile([P, 1], F32, tag="b1", name="b1")
        nc.sync.dma_start(b1_t[:], b1_d[:])
        b2_t = []
        for c in range(NDC):
            bt = wpool.tile([P, 1], F32, tag=f"b2{c}", name=f"b2{c}")
            nc.sync.dma_start(bt[:], b2_d[c])
            b2_t.append(bt)

        n_chunks = NT // CHUNK
        n_iters = CHUNK // FD
        for ci in range(n_chunks):
            csl = bass.ts(ci, CHUNK)
            qt = iopool.tile([P, NDC, CHUNK], BF, tag="qt", name="qt")
            kt = iopool.tile([P, NDC, CHUNK], BF, tag="kt", name="kt")
            post = iopool.tile([P, NDC, CHUNK], BF, tag="post", name="post")
            nc.sync.dma_start(qt[:], q_r[:, :, csl])
            nc.sync.dma_start(kt[:], k_r[:, :, csl])
            nc.sync.dma_start(post[:], pos_r[:, :, csl])
            outt = iopool.tile([P, NDC, CHUNK], BF, tag="outt", name="outt")

            for it in range(n_iters):
                tsl = bass.ts(it, FD)
                # S[e,t] = kh - qh  (accumulated in PSUM)
                S = []
                for e in range(NDC):
                    esl = bass.ts(e, P)
                    sp = pp.tile([P, FD], F32, tag="S", bufs=2, name="S")
                    nc.tensor.matmul(sp[:], wk_t[0][:, esl], kt[:, 0, tsl],
                                     start=True, stop=False)
                    nc.tensor.matmul(sp[:], wk_t[1][:, esl], kt[:, 1, tsl],
                                     start=False, stop=False)
                    nc.tensor.matmul(sp[:], wqn_t[0][:, esl], qt[:, 0, tsl],
                                     start=False, stop=False)
                    nc.tensor.matmul(sp[:], wqn_t[1][:, esl], qt[:, 1, tsl],
                                     start=False, stop=True)
                    S.append(sp)
                # v[e,t] = Wkv^T k
                V = []
                for e in range(NDC):
                    esl = bass.ts(e, P)
                    vpp = pp.tile([P, FD], F32, tag="v", bufs=2, name="V")
                    nc.tensor.matmul(vpp[:], wkv_t[0][:, esl], kt[:, 0, tsl],
                                     start=True, stop=False)
                    nc.tensor.matmul(vpp[:], wkv_t[1][:, esl], kt[:, 1, tsl],
                                     start=False, stop=True)
                    V.append(vpp)
                # attn_pre = S + pos ; vplus = v + pos  (DVE)
                ap_t = mid.tile([P, NDC, FD], BF, tag="ap", name="ap_t")
                vp_t = mid.tile([P, NDC, FD], BF, tag="vp", name="vp_t")
                for e in range(NDC):
                    nc.vector.tensor_add(ap_t[:, e, :], S[e][:], post[:, e, tsl])
                    nc.vector.tensor_add(vp_t[:, e, :], V[e][:], post[:, e, tsl])
                # h1 = relu(W1^T attn_pre + b1)
                h1p = pp.tile([P, FD], F32, tag="mlp", bufs=2, name="h1p")
                nc.tensor.matmul(h1p[:], w1_t[0][:], ap_t[:, 0, :], start=True, stop=False)
                nc.tensor.matmul(h1p[:], w1_t[1][:], ap_t[:, 1, :], start=False, stop=True)
                h1r = mid.tile([P, FD], BF, tag="h1r", name="h1r")
                nc.scalar.activation(h1r[:], h1p[:], AF.Relu, bias=b1_t[:, 0:1])
                # attn = sigmoid(W2^T h1 + b2)
                at_t = mid.tile([P, NDC, FD], BF, tag="at", name="at_t")
                for e in range(NDC):
                    esl = bass.ts(e, P)
                    a2p = pp.tile([P, FD], F32, tag="mlp", bufs=2, name="a2p")
                    nc.tensor.matmul(a2p[:], w2_t[:, esl], h1r[:], start=True, stop=True)
                    nc.scalar.activation(at_t[:, e, :], a2p[:], AF.Sigmoid,
                                         bias=b2_t[e][:, 0:1])
                # g = (v+pos)*attn  (split GpSimd / DVE, SBUF-only bf16)
                g_t = mid.tile([P, NDC, FD], BF, tag="g", name="g_t")
                nc.gpsimd.tensor_mul(g_t[:, 0, :], vp_t[:, 0, :], at_t[:, 0, :])
                nc.vector.tensor_mul(g_t[:, 1, :], vp_t[:, 1, :], at_t[:, 1, :])
                # out = Wo^T g  (bo added on host)
                for e in range(NDC):
                    esl = bass.ts(e, P)
                    xo = pp.tile([P, FD], F32, tag="xo", bufs=2, name="xo")
                    nc.tensor.matmul(xo[:], wo_t[0][:, esl], g_t[:, 0, :],
                                     start=True, stop=False)
                    nc.tensor.matmul(xo[:], wo_t[1][:, esl], g_t[:, 1, :],
                                     start=False, stop=True)
                    if e == 0:
                        nc.vector.tensor_copy(outt[:, e, tsl], xo[:])
                    else:
                        nc.scalar.copy(outt[:, e, tsl], xo[:])

            nc.sync.dma_start(out_r[:, :, csl], outt[:])

    nc.compile()
    return nc


def _get_nc():
    global _CACHED_NC
    if _CACHED_NC is None:
        _CACHED_NC = _build_nc()
    return _CACHED_NC


def _prep_in_maps(q, k, pos, Wq, Wk, Wv, W1, b1, W2, b2, Wo, bo):
    q = np.asarray(q, dtype=np.float32)
    k = np.asarray(k, dtype=np.float32)
    pos = np.asarray(pos, dtype=np.float32)

    weights = {
        "wk": np.ascontiguousarray(np.asarray(Wk, np.float32).astype(BF16)).reshape(NDC, P, DIM),
        "wqn": np.ascontiguousarray((-np.asarray(Wq, np.float32)).astype(BF16)).reshape(NDC, P, DIM),
        "wkv": np.ascontiguousarray(
            (np.asarray(Wk, np.float32) @ np.asarray(Wv, np.float32)).astype(BF16)
        ).reshape(NDC, P, DIM),
        "w1": np.ascontiguousarray(np.asarray(W1, np.float32).astype(BF16)).reshape(NDC, P, DIM // 2),
        "w2": np.ascontiguousarray(np.asarray(W2, np.float32).astype(BF16)).reshape(P, DIM),
        "wo": np.ascontiguousarray(np.asarray(Wo, np.float32).astype(BF16)).reshape(NDC, P, DIM),
        "b1": np.asarray(b1, np.float32).reshape(P, 1),
        "b2": np.asarray(b2, np.float32).reshape(NDC, P, 1),
    }

    nhalf = N // 2
    in_maps = []
    for c in range(NCORES):
        b = c // 2
        n0 = (c % 2) * nhalf
        qs = q[b, :, n0:n0 + nhalf, :].reshape(DIM, NT).astype(BF16)
        ks = k[b, :, n0:n0 + nhalf, :].reshape(DIM, NT).astype(BF16)
        ps = np.ascontiguousarray(
            pos[b, n0:n0 + nhalf].reshape(NT, DIM).T
        ).astype(BF16)
        m = dict(weights)
        m["qs"] = qs.reshape(NDC, P, NT)
        m["ks"] = ks.reshape(NDC, P, NT)
        m["poss"] = ps.reshape(NDC, P, NT)
        in_maps.append(m)
    return in_maps


def _run(in_maps, trace=False, **kwargs):
    nc = _get_nc()
    return bass_utils.run_bass_kernel_spmd(
        nc, in_maps, core_ids=list(range(NCORES)), trace=trace, **kwargs
    )


def _assemble(results, bo, mask):
    bo = np.asarray(bo, np.float32)
    out = np.empty((B, DIM, N, M), dtype=np.float32)
    nhalf = N // 2
    for c in range(NCORES):
        b = c // 2
        n0 = (c % 2) * nhalf
        r = results[c]["out"].reshape(DIM, nhalf, M).astype(np.float32)
        r += bo[:, None, None]
        out[b, :, n0:n0 + nhalf, :] = r
    mask = np.asarray(mask)
    if not np.all(mask != 0):
        # masked positions: sigmoid(-1e9)=0 -> x=0 -> out = bo
        zb, zn, zm = np.nonzero(mask[..., 0] == 0)
        out[zb, :, zn, zm] = bo[None, :]
    return out


def kernel(q, k, pos, mask, Wq, Wk, Wv, W1, b1, W2, b2, Wo, bo):
    in_maps = _prep_in_maps(q, k, pos, Wq, Wk, Wv, W1, b1, W2, b2, Wo, bo)
    res = _run(in_maps)
    return _assemble(res.results, bo, mask)


def install_profile_hook():
    """Register the axon NTFF profiling hook (antenv.axon_hooks shim) so
    run_bass_kernel_spmd(trace=True) yields exec_time_ns + perfetto trace."""
    import types

    try:
        import antenv.axon_hooks  # noqa: F401
        return True
    except ImportError:
        pass
    try:
        from trn_agent_boot.trn_boot import _ntff_profile_via_ctypes
    except ImportError:
        return False
    hook = _ntff_profile_via_ctypes("/opt/axon/libaxon_pjrt.so")
    if hook is None:
        return False
    mod = types.ModuleType("antenv.axon_hooks")
    mod.get_axon_ntff_profile_hook = lambda: hook
    mod.set_axon_ntff_profile_hook = lambda h: None
    import antenv

    sys.modules["antenv.axon_hooks"] = mod
    antenv.axon_hooks = mod
    # artifact upload has no share reachable from this container
    bass_utils.upload_artifacts = lambda tmpdir: tmpdir
    return True


# revision 4
# speedup vs baseline: 1.2881x; 1.1225x over previous
"""Trainium2 Bass kernel for nn_Attention_21088289423660 (sparse_attention).

Reference computation (per token t = (b, n, m), feature dim D=256):
    kh = Wk^T k_t                  (feature-major: [e, t])
    qh = Wq^T q_t
    v  = Wv^T kh = (Wk Wv)^T k_t   <- folded on host: Wkv = Wk @ Wv
    S  = kh - qh + pos_t           <- Wqn = -Wq accumulated in PSUM
    attn = sigmoid(W2^T relu(W1^T S + b1) + b2)      (mask is all-ones)
    out  = Wo^T ((v + pos_t) * attn) (+ bo on host)  (already feature-major)

Sharding: data-parallel over 8 cores; core c handles batch b=c//2 and
N-half (c%2) -> 16384 tokens/core, weights replicated.

Compute dtype: bf16 (PSUM accumulation fp32), device output bf16,
host adds bo and widens to fp32.
"""

import os
import sys

for _p in (
    "/root/.axon_site",
    "/root/.axon_site/_ro/trn_rl_repo",
    "/root/.axon_site/_ro/pypackages",
    "/opt/trn_rl_repo",
):
    if os.path.isdir(_p) and _p not in sys.path:
        sys.path.append(_p)

import numpy as np
import ml_dtypes
from contextlib import ExitStack

import concourse.bass as bass
import concourse.tile as tile
import concourse.bacc as bacc
from concourse import mybir
from concourse import bass_utils

BF16 = ml_dtypes.bfloat16

B, DIM, N, M = 4, 256, 4096, 8
NCORES = 8
NT = (B * N * M) // NCORES          # tokens per core = 16384
P = 128                              # partitions
NDC = DIM // P                       # d-chunks = 2
CHUNK = 4096                         # tokens per DMA chunk
FD = 512                             # tokens per matmul tile
F32 = mybir.dt.float32
BF = mybir.dt.bfloat16

_CACHED_NC = None


def _build_nc():
    """Build and compile the per-core Bass program (SPMD, identical on all cores)."""
    nc = bacc.Bacc("TRN2", target_bir_lowering=False, debug=False)

    # DRAM I/O (per-core shapes)
    q_d = nc.dram_tensor("qs", (NDC, P, NT), BF, kind="ExternalInput").ap()
    k_d = nc.dram_tensor("ks", (NDC, P, NT), BF, kind="ExternalInput").ap()
    pos_d = nc.dram_tensor("poss", (NDC, P, NT), BF, kind="ExternalInput").ap()
    wk_d = nc.dram_tensor("wk", (NDC, P, DIM), BF, kind="ExternalInput").ap()
    wqn_d = nc.dram_tensor("wqn", (NDC, P, DIM), BF, kind="ExternalInput").ap()
    wkv_d = nc.dram_tensor("wkv", (NDC, P, DIM), BF, kind="ExternalInput").ap()
    w1_d = nc.dram_tensor("w1", (NDC, P, DIM // 2), BF, kind="ExternalInput").ap()
    w2_d = nc.dram_tensor("w2", (P, DIM), BF, kind="ExternalInput").ap()
    wo_d = nc.dram_tensor("wo", (NDC, P, DIM), BF, kind="ExternalInput").ap()
    b1_d = nc.dram_tensor("b1", (P, 1), F32, kind="ExternalInput").ap()
    b2_d = nc.dram_tensor("b2", (NDC, P, 1), F32, kind="ExternalInput").ap()
    out_d = nc.dram_tensor("out", (NDC, P, NT), BF, kind="ExternalOutput").ap()

    q_r = q_d.rearrange("c p t -> p c t")
    k_r = k_d.rearrange("c p t -> p c t")
    pos_r = pos_d.rearrange("c p t -> p c t")
    out_r = out_d.rearrange("c p t -> p c t")

    AF = mybir.ActivationFunctionType

    with tile.TileContext(nc) as tc, ExitStack() as ctx:
        wpool = ctx.enter_context(tc.tile_pool(name="wpool", bufs=1))
        iopool = ctx.enter_context(tc.tile_pool(name="iopool", bufs=2))
        mid = ctx.enter_context(tc.tile_pool(name="mid", bufs=3))
        pp = ctx.enter_context(tc.tile_pool(name="pp", bufs=1, space="PSUM"))

        # --- weights / biases resident in SBUF ---
        wk_t, wqn_t, wkv_t, wo_t, w1_t = [], [], [], [], []
        for c in range(NDC):
            wt = wpool.tile([P, DIM], BF, tag=f"wk{c}", name=f"wk{c}")
            nc.sync.dma_start(wt[:], wk_d[c])
            wk_t.append(wt)
            wt = wpool.tile([P, DIM], BF, tag=f"wqn{c}", name=f"wqn{c}")
            nc.sync.dma_start(wt[:], wqn_d[c])
            wqn_t.append(wt)
            wt = wpool.tile([P, DIM], BF, tag=f"wkv{c}", name=f"wkv{c}")
            nc.sync.dma_start(wt[:], wkv_d[c])
            wkv_t.append(wt)
            wt = wpool.tile([P, DIM], BF, tag=f"wo{c}", name=f"wo{c}")
            nc.sync.dma_start(wt[:], wo_d[c])
            wo_t.append(wt)
            wt = wpool.tile([P, DIM // 2], BF, tag=f"w1{c}", name=f"w1{c}")
            nc.sync.dma_start(wt[:], w1_d[c])
            w1_t.append(wt)
        w2_t = wpool.tile([P, DIM], BF, tag="w2", name="w2")
        nc.sync.dma_start(w2_t[:], w2_d[:])
        b1_t = wpool.tile([P, 1], F32, tag="b1", name="b1")
        nc.sync.dma_start(b1_t[:], b1_d[:])
        b2_t = []
        for c in range(NDC):
            bt = wpool.tile([P, 1], F32, tag=f"b2{c}", name=f"b2{c}")
            nc.sync.dma_start(bt[:], b2_d[c])
            b2_t.append(bt)

        n_chunks = NT // CHUNK
        n_iters = CHUNK // FD
        for ci in range(n_chunks):
            csl = bass.ts(ci, CHUNK)
            qt = iopool.tile([P, NDC, CHUNK], BF, tag="qt", name="qt")
            kt = iopool.tile([P, NDC, CHUNK], BF, tag="kt", name="kt")
            post = iopool.tile([P, NDC, CHUNK], BF, tag="post", name="post")
            if ci == 0:
                # stage the first tiles in small pieces so PE starts early
                for sl in (slice(0, FD), slice(FD, 2 * FD), slice(2 * FD, CHUNK)):
                    nc.sync.dma_start(kt[:, :, sl], k_r[:, :, sl])
                    nc.sync.dma_start(qt[:, :, sl], q_r[:, :, sl])
                    nc.sync.dma_start(post[:, :, sl], pos_r[:, :, sl])
            else:
                nc.sync.dma_start(qt[:], q_r[:, :, csl])
                nc.sync.dma_start(kt[:], k_r[:, :, csl])
                nc.sync.dma_start(post[:], pos_r[:, :, csl])
            outt = iopool.tile([P, NDC, CHUNK], BF, tag="outt", name="outt")

            prev = None

            def emit_out(pv):
                g_p, tsl_p = pv
                for e in range(NDC):
                    esl = bass.ts(e, P)
                    xo = pp.tile([P, FD], F32, tag="xo", bufs=2, name="xo")
                    nc.tensor.matmul(xo[:], wo_t[0][:, esl], g_p[:, 0, :],
                                     start=True, stop=False)
                    nc.tensor.matmul(xo[:], wo_t[1][:, esl], g_p[:, 1, :],
                                     start=False, stop=True)
                    nc.scalar.copy(outt[:, e, tsl_p], xo[:])

            for it in range(n_iters):
                tsl = bass.ts(it, FD)
                # S[e,t] = kh - qh  (accumulated in PSUM)
                S = []
                for e in range(NDC):
                    esl = bass.ts(e, P)
                    sp = pp.tile([P, FD], F32, tag="S", bufs=2, name="S")
                    nc.tensor.matmul(sp[:], wk_t[0][:, esl], kt[:, 0, tsl],
                                     start=True, stop=False)
                    nc.tensor.matmul(sp[:], wk_t[1][:, esl], kt[:, 1, tsl],
                                     start=False, stop=False)
                    nc.tensor.matmul(sp[:], wqn_t[0][:, esl], qt[:, 0, tsl],
                                     start=False, stop=False)
                    nc.tensor.matmul(sp[:], wqn_t[1][:, esl], qt[:, 1, tsl],
                                     start=False, stop=True)
                    S.append(sp)
                # v[e,t] = Wkv^T k
                V = []
                for e in range(NDC):
                    esl = bass.ts(e, P)
                    vpp = pp.tile([P, FD], F32, tag="v", bufs=2, name="V")
                    nc.tensor.matmul(vpp[:], wkv_t[0][:, esl], kt[:, 0, tsl],
                                     start=True, stop=False)
                    nc.tensor.matmul(vpp[:], wkv_t[1][:, esl], kt[:, 1, tsl],
                                     start=False, stop=True)
                    V.append(vpp)
                # attn_pre = S + pos ; vplus = v + pos  (DVE)
                ap_t = mid.tile([P, NDC, FD], BF, tag="ap", name="ap_t")
                vp_t = mid.tile([P, NDC, FD], BF, tag="vp", name="vp_t")
                for e in range(NDC):
                    nc.vector.tensor_add(ap_t[:, e, :], S[e][:], post[:, e, tsl])
                    nc.vector.tensor_add(vp_t[:, e, :], V[e][:], post[:, e, tsl])
                # h1 = relu(W1^T attn_pre + b1)
                h1p = pp.tile([P, FD], F32, tag="mlp", bufs=2, name="h1p")
                nc.tensor.matmul(h1p[:], w1_t[0][:], ap_t[:, 0, :], start=True, stop=False)
                nc.tensor.matmul(h1p[:], w1_t[1][:], ap_t[:, 1, :], start=False, stop=True)
                h1r = mid.tile([P, FD], BF, tag="h1r", name="h1r")
                nc.scalar.activation(h1r[:], h1p[:], AF.Relu, bias=b1_t[:, 0:1])
                if prev is not None:
                    emit_out(prev)
                    prev = None
                # attn = sigmoid(W2^T h1 + b2)
                at_t = mid.tile([P, NDC, FD], BF, tag="at", name="at_t")
                for e in range(NDC):
                    esl = bass.ts(e, P)
                    a2p = pp.tile([P, FD], F32, tag="mlp", bufs=2, name="a2p")
                    nc.tensor.matmul(a2p[:], w2_t[:, esl], h1r[:], start=True, stop=True)
                    nc.scalar.activation(at_t[:, e, :], a2p[:], AF.Sigmoid,
                                         bias=b2_t[e][:, 0:1])
                # g = (v+pos)*attn  (split GpSimd / DVE, SBUF-only bf16)
                g_t = mid.tile([P, NDC, FD], BF, tag="g", name="g_t")
                nc.gpsimd.tensor_mul(g_t[:, 0, :], vp_t[:, 0, :], at_t[:, 0, :])
                nc.vector.tensor_mul(g_t[:, 1, :], vp_t[:, 1, :], at_t[:, 1, :])
                prev = (g_t, tsl)

            emit_out(prev)
            nc.sync.dma_start(out_r[:, :, csl], outt[:])

    nc.compile()
    return nc


def _get_nc():
    global _CACHED_NC
    if _CACHED_NC is None:
        _CACHED_NC = _build_nc()
    return _CACHED_NC


def _prep_in_maps(q, k, pos, Wq, Wk, Wv, W1, b1, W2, b2, Wo, bo):
    q = np.asarray(q, dtype=np.float32)
    k = np.asarray(k, dtype=np.float32)
    pos = np.asarray(pos, dtype=np.float32)

    weights = {
        "wk": np.ascontiguousarray(np.asarray(Wk, np.float32).astype(BF16)).reshape(NDC, P, DIM),
        "wqn": np.ascontiguousarray((-np.asarray(Wq, np.float32)).astype(BF16)).reshape(NDC, P, DIM),
        "wkv": np.ascontiguousarray(
            (np.asarray(Wk, np.float32) @ np.asarray(Wv, np.float32)).astype(BF16)
        ).reshape(NDC, P, DIM),
        "w1": np.ascontiguousarray(np.asarray(W1, np.float32).astype(BF16)).reshape(NDC, P, DIM // 2),
        "w2": np.ascontiguousarray(np.asarray(W2, np.float32).astype(BF16)).reshape(P, DIM),
        "wo": np.ascontiguousarray(np.asarray(Wo, np.float32).astype(BF16)).reshape(NDC, P, DIM),
        "b1": np.asarray(b1, np.float32).reshape(P, 1),
        "b2": np.asarray(b2, np.float32).reshape(NDC, P, 1),
    }

    nhalf = N // 2
    in_maps = []
    for c in range(NCORES):
        b = c // 2
        n0 = (c % 2) * nhalf
        qs = q[b, :, n0:n0 + nhalf, :].reshape(DIM, NT).astype(BF16)
        ks = k[b, :, n0:n0 + nhalf, :].reshape(DIM, NT).astype(BF16)
        ps = np.ascontiguousarray(
            pos[b, n0:n0 + nhalf].reshape(NT, DIM).T
        ).astype(BF16)
        m = dict(weights)
        m["qs"] = qs.reshape(NDC, P, NT)
        m["ks"] = ks.reshape(NDC, P, NT)
        m["poss"] = ps.reshape(NDC, P, NT)
        in_maps.append(m)
    return in_maps


def _run(in_maps, trace=False, **kwargs):
    nc = _get_nc()
    return bass_utils.run_bass_kernel_spmd(
        nc, in_maps, core_ids=list(range(NCORES)), trace=trace, **kwargs
    )


def _assemble(results, bo, mask):
    bo = np.asarray(bo, np.float32)
    out = np.empty((B, DIM, N, M), dtype=np.float32)
    nhalf = N // 2
    for c in range(NCORES):
        b = c // 2
        n0 = (c % 2) * nhalf
        r = results[c]["out"].reshape(DIM, nhalf, M).astype(np.float32)
        r += bo[:, None, None]
        out[b, :, n0:n0 + nhalf, :] = r
    mask = np.asarray(mask)
    if not np.all(mask != 0):
        # masked positions: sigmoid(-1e9)=0 -> x=0 -> out = bo
        zb, zn, zm = np.nonzero(mask[..., 0] == 0)
        out[zb, :, zn, zm] = bo[None, :]
    return out


def kernel(q, k, pos, mask, Wq, Wk, Wv, W1, b1, W2, b2, Wo, bo):
    in_maps = _prep_in_maps(q, k, pos, Wq, Wk, Wv, W1, b1, W2, b2, Wo, bo)
    res = _run(in_maps)
    return _assemble(res.results, bo, mask)


def install_profile_hook():
    """Register the axon NTFF profiling hook (antenv.axon_hooks shim) so
    run_bass_kernel_spmd(trace=True) yields exec_time_ns + perfetto trace."""
    import types

    try:
        import antenv.axon_hooks  # noqa: F401
        return True
    except ImportError:
        pass
    try:
        from trn_agent_boot.trn_boot import _ntff_profile_via_ctypes
    except ImportError:
        return False
    hook = _ntff_profile_via_ctypes("/opt/axon/libaxon_pjrt.so")
    if hook is None:
        return False
    mod = types.ModuleType("antenv.axon_hooks")
    mod.get_axon_ntff_profile_hook = lambda: hook
    mod.set_axon_ntff_profile_hook = lambda h: None
    import antenv

    sys.modules["antenv.axon_hooks"] = mod
    antenv.axon_hooks = mod
    # artifact upload has no share reachable from this container
    bass_utils.upload_artifacts = lambda tmpdir: tmpdir
    return True
